# revision 28
# baseline (speedup 1.0000x reference)
"""AgentSelfAttention1d Trainium2 kernel (v2).

Per batch b (one NeuronCore each):
    xt = x[b].T                       # [L=4096, D=512]
    q/k/v = xt @ W{q,k,v}.T + b       # [L, D]
    a  = AdaptiveAvgPool(q) -> [P=128, D]
    c  = softmax(a @ k.T, -1) @ v     # [P, D]
    r  = softmax(q @ a.T, -1) @ c     # [L, D]
    out[b] = r.T                      # [D, L]

Restructuring (all projections folded into host-precomputed weight
products; everything channel-first on chip):
    xp[c,p]   = seg-sum of x over 32-wide windows      (via tiny PE matmuls
                against a one-hot segment indicator, from the x.T tiles)
    H[e,p]    = MKs[c,e]^T-contract xp + vk[e],  MKs = (Wq^T Wk)/32,
                vk = Wk^T bq          (S1[p,l] = sum_e H[e,p] x[e,l])
    G[e,p]    = MQs-contract xp + vq[e],         MQs = (Wq^T Wq)/32
    hq[p]     = (xp^T (Wq^T bq))/32 + |bq|^2     (S2T[p,l] = G-part + hq)
    E1        = exp(S1 - 10)  bf16; rowsum via activation accumulator
    M1[p,e]   = E1 @ x.T   (E1 transposed by the DMA xbar engine)
    cbv[p,d]  = (M1 @ Wv^T) / rowsum1 + bv
    E2        = exp(S2T - 40) fp16 (unnormalized)
    out[l,d]  = (sum_p E2[p,l] cbv[p,d]) / colsum2[l]
                -- contraction over p needs NO transpose (p is already on
                partitions); colsum2 rides along as N=2 ones-matmuls and is
                applied as a per-partition scale on the output copies.
    Output written [L, D] fp16; host transposes/upcasts to [D, L] f32.

Softmaxes use constant logit shifts (|S1|~21, |S2|~42) instead of max
subtraction; exp stays in range (bf16 for E1, fp16 for E2).
"""

import numpy as np
import ml_dtypes

import concourse.bass as bass
import concourse.mybir as mybir
import concourse.tile as tile
from concourse import bacc
from concourse.bass_utils import run_bass_kernel_spmd

F32 = mybir.dt.float32
F16 = mybir.dt.float16
BF16 = mybir.dt.bfloat16

B, D, L, P = 8, 512, 4096, 128
KT = D // 128      # 4 contraction tiles of 128
NCH = L // 512     # 8 l-chunks of 512
NLT = L // 128     # 32 l-tiles of 128
SEG = L // P       # 32: pool segment length
SHIFT1 = 10.0
SHIFT2 = 40.0

_CACHE = {}


def build():
    nc = bacc.Bacc(target_bir_lowering=False, trn_type="TRN2")
    X = nc.dram_tensor("x", [D, L], F16, kind="ExternalInput")
    MKS = nc.dram_tensor("mks", [D, D], F16, kind="ExternalInput")   # (Wq^T Wk)/32 [c,e]
    MQS = nc.dram_tensor("mqs", [D, D], F16, kind="ExternalInput")   # (Wq^T Wq)/32 [c,e]
    WVT = nc.dram_tensor("wvt", [D, D], BF16, kind="ExternalInput")  # Wv^T [e,d]
    IDN = nc.dram_tensor("ident", [128, 128], F16, kind="ExternalInput")
    INDS = nc.dram_tensor("inds", [128, 4], BF16, kind="ExternalInput")  # l -> l//32 one-hot
    ONES2 = nc.dram_tensor("ones2", [128, 2], BF16, kind="ExternalInput")
    VK = nc.dram_tensor("vk", [D], F32, kind="ExternalInput")        # Wk^T bq
    VQ = nc.dram_tensor("vq", [D], F32, kind="ExternalInput")        # Wq^T bq
    VQ2 = nc.dram_tensor("vq2", [D, 2], F16, kind="ExternalInput")   # [(Wq^T bq)/32, 0]
    BCONST = nc.dram_tensor("bconst", [128], F32, kind="ExternalInput")  # |bq|^2-SHIFT2
    BVB = nc.dram_tensor("bvb", [128, D], BF16, kind="ExternalInput")     # bv bcast
    OUT = nc.dram_tensor("out", [L, D], F16, kind="ExternalOutput")

    from contextlib import ExitStack
    with nc.allow_low_precision("16-bit matmul operands"), \
         tile.TileContext(nc) as tc, ExitStack() as stack:
        sb = stack.enter_context(tc.tile_pool(name="sb", bufs=1))
        e1p = stack.enter_context(tc.tile_pool(name="e1p", bufs=3))
        e1tp = stack.enter_context(tc.tile_pool(name="e1tp", bufs=3))
        e2p = stack.enter_context(tc.tile_pool(name="e2p", bufs=3))
        outp = stack.enter_context(tc.tile_pool(name="outp", bufs=3))
        iv2p = stack.enter_context(tc.tile_pool(name="iv2p", bufs=3))
        # PSUM (8 banks): xT 2 + xp 1 + hg 1 | pass1: s1 2 + m1 1 |
        # pass2: s2 2 + out 3 + rs2 1
        psA = stack.enter_context(tc.tile_pool(name="psA", bufs=3, space="PSUM"))
        psB = stack.enter_context(tc.tile_pool(name="psB", bufs=4, space="PSUM"))
        psC = stack.enter_context(tc.tile_pool(name="psC", bufs=1, space="PSUM"))

        # ---- ACT table warmup ------------------------------------------------
        warm = sb.tile([128, 1], F32)
        nc.vector.memset(warm, 0.0)
        nc.scalar.activation(out=warm, in_=warm,
                             func=mybir.ActivationFunctionType.Exp,
                             bias=warm, scale=1.0)

        # ---- input DMAs (order = DMA_ENGINES order) --------------------------
        ident = sb.tile([128, 128], F16)
        nc.gpsimd.dma_start(out=ident, in_=IDN[:, :])        # SWDGE, off HWDGE
        inds = sb.tile([128, 4], BF16)
        nc.gpsimd.dma_start(out=inds, in_=INDS[:, :])
        x_sb = sb.tile([128, KT, L], F16)
        xr = X.rearrange("(k p) l -> p k l", p=128)
        for ch in range(NCH):
            nc.sync.dma_start(out=x_sb[:, :, bass.ts(ch, 512)],
                              in_=xr[:, :, bass.ts(ch, 512)])
        mks = sb.tile([128, KT, D], F16)
        nc.sync.dma_start(out=mks, in_=MKS.rearrange("(k p) e -> p k e", p=128))
        vk = sb.tile([128, KT], F32)
        nc.gpsimd.dma_start(out=vk, in_=VK.rearrange("(k p) -> p k", p=128))
        vq = sb.tile([128, KT], F32)
        nc.gpsimd.dma_start(out=vq, in_=VQ.rearrange("(k p) -> p k", p=128))
        vq2 = sb.tile([128, KT, 2], F16)
        nc.gpsimd.dma_start(out=vq2, in_=VQ2.rearrange("(k p) t -> p k t", p=128))
        bconst = sb.tile([128, 1], F32)
        nc.gpsimd.dma_start(out=bconst, in_=BCONST.rearrange("(p o) -> p o", o=1))
        ones2 = sb.tile([128, 2], BF16)
        nc.gpsimd.dma_start(out=ones2, in_=ONES2[:, :])
        bvb = sb.tile([128, D], BF16)
        nc.gpsimd.dma_start(out=bvb, in_=BVB[:, :])
        mqs = sb.tile([128, KT, D], F16)
        nc.sync.dma_start(out=mqs, in_=MQS.rearrange("(k p) e -> p k e", p=128))
        wvt = sb.tile([128, KT, D], BF16)
        nc.sync.dma_start(out=wvt, in_=WVT.rearrange("(k p) e -> p k e", p=128))
        sh1 = sb.tile([128, 1], F32)
        nc.vector.memset(sh1, -SHIFT1)

        # ---- x.T tiles (PE transpose) + pooling (tiny PE matmuls) ------------
        xt = sb.tile([128, NLT, D], BF16)
        xpps = psC.tile([128, KT, 128], F32, tag="c")

        xp = sb.tile([128, KT, 128], F16)

        def pool_mm(jp):
            for h in range(2):
                j = 2 * jp + h
                for t in range(KT):
                    nc.tensor.matmul(xpps[:, t, 4 * j:4 * j + 4],
                                     xt[:, j, bass.ts(t, 128)], inds,
                                     start=True, stop=True)
            if jp % 2 == 1:
                ch = (jp - 1) // 2
                nc.scalar.copy(xp[:, :, 16 * ch:16 * ch + 16],
                               xpps[:, :, 16 * ch:16 * ch + 16])

        for jp in range(NLT // 2):
            tp = psB.tile([128, 2, 512], F16, tag="b")
            for h in range(2):
                j = 2 * jp + h
                for k in range(KT):
                    nc.tensor.transpose(tp[:, h, bass.ts(k, 128)],
                                        x_sb[:, k, bass.ts(j, 128)], ident)
            nc.vector.tensor_copy(xt[:, 2 * jp:2 * jp + 2, :], tp)
            if jp >= 4:
                pool_mm(jp - 4)
        for jq in range(NLT // 2 - 4, NLT // 2):
            pool_mm(jq)
        # ---- H, G, hq (copies on DVE to keep Act free) -----------------------
        h_sb = sb.tile([128, KT, 128], F16)
        hps = psA.tile([128, KT, 128], F32, tag="a")
        for et in range(KT):
            for ck in range(KT):
                nc.tensor.matmul(hps[:, et, :],
                                 mks[:, ck, bass.ts(et, 128)], xp[:, ck, :],
                                 start=(ck == 0), stop=(ck == KT - 1))
        for et in range(KT):
            nc.scalar.activation(out=h_sb[:, et, :], in_=hps[:, et, :],
                                 func=mybir.ActivationFunctionType.Identity,
                                 bias=vk[:, et:et + 1], scale=1.0)
        hqps = psA.tile([128, 2], F32, tag="a")
        for ck in range(KT):
            nc.tensor.matmul(hqps, xp[:, ck, :], vq2[:, ck, :],
                             start=(ck == 0), stop=(ck == KT - 1))
        hq = sb.tile([128, 1], F32)
        nc.scalar.activation(out=hq, in_=hqps[:, 0:1],
                             func=mybir.ActivationFunctionType.Identity,
                             bias=bconst, scale=1.0)
        g_sb = sb.tile([128, KT, 128], F16)
        gps = psA.tile([128, KT, 128], F32, tag="a")
        for et in range(KT):
            for ck in range(KT):
                nc.tensor.matmul(gps[:, et, :],
                                 mqs[:, ck, bass.ts(et, 128)], xp[:, ck, :],
                                 start=(ck == 0), stop=(ck == KT - 1))
        for et in range(KT):
            nc.scalar.activation(out=g_sb[:, et, :], in_=gps[:, et, :],
                                 func=mybir.ActivationFunctionType.Identity,
                                 bias=vq[:, et:et + 1], scale=1.0)

        # ---- pass 1: S1 -> E1 -> (xbar) E1T -> M1, software-pipelined --------
        # M1 for chunk a is issued after S1 for chunk a+2, so the PE never
        # stalls on the exp + xbar-transpose round trip.
        rs1 = sb.tile([128, NCH], F32)
        m1tps = []
        for ek in range(KT):
            m1b = psB.tile([128, 128], F32, tag="b")
            m1tps.append(m1b)
        e1ts = []

        def s1_stage(a):
            s1 = psA.tile([128, 512], F32, tag="a")
            for k in range(KT):
                nc.tensor.matmul(s1, h_sb[:, k, :], x_sb[:, k, bass.ts(a, 512)],
                                 start=(k == 0), stop=(k == KT - 1))
            e1 = e1p.tile([128, 512], BF16, tag="e1")
            nc.scalar.activation(out=e1, in_=s1,
                                 func=mybir.ActivationFunctionType.Exp,
                                 bias=sh1, scale=1.0,
                                 accum_out=rs1[:, a:a + 1])
            e1t = e1tp.tile([128, 4, 128], BF16, tag="e1t")
            nc.sync.dma_start_transpose(e1t, e1)
            e1ts.append(e1t)

        def m1_stage(a):
            e1t = e1ts[a]
            for u in range(4):
                j = 4 * a + u
                for ek in range(KT):
                    nc.tensor.matmul(m1tps[ek],
                                     xt[:, j, bass.ts(ek, 128)], e1t[:, u, :],
                                     start=(j == 0), stop=(j == NLT - 1))

        s1_stage(0)
        s1_stage(1)
        for a in range(NCH):
            if a + 2 < NCH:
                s1_stage(a + 2)
            m1_stage(a)

        # ---- pass 2: S2 -> E2 -> out = (E2^T cbv) * inv2 ---------------------
        # ident (fp16) reused as transpose stationary; e2 slices are the
        # stationary lhs of the output matmuls (contraction over p needs no
        # transpose), ones2 column rides along for colsum2.
        or_ = OUT.rearrange("(c j p) d -> c p j d", j=4, p=128)
        e2s = []

        def s2_stage(a):
            s2 = psA.tile([128, 512], F32, tag="a")
            for k in range(KT):
                nc.tensor.matmul(s2, g_sb[:, k, :], x_sb[:, k, bass.ts(a, 512)],
                                 start=(k == 0), stop=(k == KT - 1))
            e2 = e2p.tile([128, 512], BF16, tag="e2")
            nc.scalar.activation(out=e2, in_=s2,
                                 func=mybir.ActivationFunctionType.Exp,
                                 bias=hq, scale=1.0)
            e2s.append(e2)

        inv2s = []

        def rs2_stage(a):
            e2 = e2s[a]
            rsps = psC.tile([128, 4, 2], F32, tag="c")
            for u in range(4):
                nc.tensor.matmul(rsps[:, u, :], e2[:, bass.ts(u, 128)], ones2,
                                 start=True, stop=True)
            inv2 = iv2p.tile([128, 4], F32, tag="iv2")
            nc.vector.reciprocal(inv2, rsps[:, :, 0])
            inv2s.append(inv2)

        def out_stage(a):
            e2 = e2s[a]
            inv2 = inv2s[a]
            o_sb = outp.tile([128, 4, D], F16, tag="o")
            for u in range(4):
                ops = psB.tile([128, D], F32, tag="b")
                nc.tensor.matmul(ops, e2[:, bass.ts(u, 128)], cbv,
                                 start=True, stop=True)
                on_act = (u == 0) or (u == 2 and a % 4 != 1)
                if on_act:
                    nc.scalar.activation(
                        out=o_sb[:, u, :], in_=ops,
                        func=mybir.ActivationFunctionType.Identity,
                        bias=0.0, scale=inv2[:, u:u + 1])
                else:
                    nc.vector.tensor_scalar_mul(o_sb[:, u, :], ops,
                                                inv2[:, u:u + 1])
            nc.sync.dma_start(out=or_[a], in_=o_sb)

        s2_stage(0)
        s2_stage(1)
        rs2_stage(0)
        rsum1 = sb.tile([128, 1], F32)
        nc.vector.reduce_sum(out=rsum1, in_=rs1, axis=mybir.AxisListType.X)
        inv1 = sb.tile([128, 1], F32)
        nc.vector.reciprocal(inv1, rsum1)

        # ---- c = (M1 @ Wv^T)/rowsum1 + bv ------------------------------------
        m1t = sb.tile([128, KT, 128], BF16)
        for ek in range(KT):
            if ek % 2 == 0:
                nc.scalar.copy(m1t[:, ek, :], m1tps[ek])
            else:
                nc.vector.tensor_copy(m1t[:, ek, :], m1tps[ek])
        cps = psA.tile([128, D], F32, tag="a")
        for i in range(KT):
            nc.tensor.matmul(cps, m1t[:, i, :], wvt[:, i, :],
                             start=(i == 0), stop=(i == KT - 1))
        cbv = sb.tile([128, D], BF16)
        nc.vector.scalar_tensor_tensor(out=cbv, in0=cps, scalar=inv1, in1=bvb,
                                       op0=mybir.AluOpType.mult,
                                       op1=mybir.AluOpType.add)

        for a in range(NCH):
            if a + 2 < NCH:
                s2_stage(a + 2)
            if a + 1 < NCH:
                rs2_stage(a + 1)
            out_stage(a)

    nc.compile()
    return nc


def _host_inputs(x, Wq, bq, Wk, bk, Wv, bv):
    del bk  # stage-1 softmax is invariant to the k-projection bias
    Wq = np.asarray(Wq, dtype=np.float32)
    Wk = np.asarray(Wk, dtype=np.float32)
    Wv = np.asarray(Wv, dtype=np.float32)
    bq = np.asarray(bq, dtype=np.float32)
    bv = np.asarray(bv, dtype=np.float32)
    bf16 = ml_dtypes.bfloat16
    inds = np.zeros((128, 4), dtype=np.float32)
    inds[np.arange(128), np.arange(128) // SEG] = 1.0
    ones2 = np.zeros((128, 2), dtype=np.float32)
    ones2[:, 0] = 1.0
    common = {
        "mks": ((Wq.T @ Wk) / SEG).astype(np.float16),
        "mqs": ((Wq.T @ Wq) / SEG).astype(np.float16),
        "wvt": np.ascontiguousarray(Wv.T).astype(bf16),
        "ident": np.eye(128, dtype=np.float16),
        "inds": inds.astype(bf16),
        "ones2": ones2.astype(bf16),
        "vk": (Wk.T @ bq).astype(np.float32),
        "vq": (Wq.T @ bq).astype(np.float32),
        "vq2": np.stack([(Wq.T @ bq) / SEG, np.zeros(D, np.float32)],
                        axis=1).astype(np.float16),
        "bconst": np.full(128, float(bq @ bq) - SHIFT2, dtype=np.float32),
        "bvb": np.tile(bv[None, :], (128, 1)).astype(bf16),
    }
    maps = []
    for b in range(B):
        m = dict(common)
        m["x"] = np.ascontiguousarray(x[b]).astype(np.float16)
        maps.append(m)
    return maps


def kernel(x, Wq, bq, Wk, bk, Wv, bv):
    x = np.asarray(x, dtype=np.float32)
    if "nc" not in _CACHE:
        _CACHE["nc"] = build()
    nc = _CACHE["nc"]
    in_maps = _host_inputs(x, Wq, bq, Wk, bk, Wv, bv)
    res = run_bass_kernel_spmd(nc, in_maps, core_ids=list(range(B)))
    out = np.empty((B, D, L), dtype=np.float32)
    for b in range(B):
        out[b] = np.asarray(res.results[b]["out"]).astype(np.float32).T
    return out


# revision 29
# speedup vs baseline: 1.0279x; 1.0279x over previous
"""AgentSelfAttention1d Trainium2 kernel (v2).

Per batch b (one NeuronCore each):
    xt = x[b].T                       # [L=4096, D=512]
    q/k/v = xt @ W{q,k,v}.T + b       # [L, D]
    a  = AdaptiveAvgPool(q) -> [P=128, D]
    c  = softmax(a @ k.T, -1) @ v     # [P, D]
    r  = softmax(q @ a.T, -1) @ c     # [L, D]
    out[b] = r.T                      # [D, L]

Restructuring (all projections folded into host-precomputed weight
products; everything channel-first on chip):
    xp[c,p]   = seg-sum of x over 32-wide windows      (via tiny PE matmuls
                against a one-hot segment indicator, from the x.T tiles)
    H[e,p]    = MKs[c,e]^T-contract xp + vk[e],  MKs = (Wq^T Wk)/32,
                vk = Wk^T bq          (S1[p,l] = sum_e H[e,p] x[e,l])
    G[e,p]    = MQs-contract xp + vq[e],         MQs = (Wq^T Wq)/32
    hq[p]     = (xp^T (Wq^T bq))/32 + |bq|^2     (S2T[p,l] = G-part + hq)
    E1        = exp(S1 - 10)  bf16; rowsum via activation accumulator
    M1[p,e]   = E1 @ x.T   (E1 transposed by the DMA xbar engine)
    cbv[p,d]  = (M1 @ Wv^T) / rowsum1 + bv
    E2        = exp(S2T - 40) fp16 (unnormalized)
    out[l,d]  = (sum_p E2[p,l] cbv[p,d]) / colsum2[l]
                -- contraction over p needs NO transpose (p is already on
                partitions); colsum2 rides along as N=2 ones-matmuls and is
                applied as a per-partition scale on the output copies.
    Output written [L, D] fp16; host transposes/upcasts to [D, L] f32.

Softmaxes use constant logit shifts (|S1|~21, |S2|~42) instead of max
subtraction; exp stays in range (bf16 for E1, fp16 for E2).
"""

import numpy as np
import ml_dtypes

import concourse.bass as bass
import concourse.mybir as mybir
import concourse.tile as tile
from concourse import bacc
from concourse.bass_utils import run_bass_kernel_spmd

F32 = mybir.dt.float32
F16 = mybir.dt.float16
BF16 = mybir.dt.bfloat16

B, D, L, P = 8, 512, 4096, 128
KT = D // 128      # 4 contraction tiles of 128
NCH = L // 512     # 8 l-chunks of 512
NLT = L // 128     # 32 l-tiles of 128
SEG = L // P       # 32: pool segment length
SHIFT1 = 10.0
SHIFT2 = 40.0

_CACHE = {}


def build():
    nc = bacc.Bacc(target_bir_lowering=False, trn_type="TRN2")
    X = nc.dram_tensor("x", [D, L], F16, kind="ExternalInput")
    MKS = nc.dram_tensor("mks", [D, D], F16, kind="ExternalInput")   # (Wq^T Wk)/32 [c,e]
    MQS = nc.dram_tensor("mqs", [D, D], F16, kind="ExternalInput")   # (Wq^T Wq)/32 [c,e]
    WVT = nc.dram_tensor("wvt", [D, D], BF16, kind="ExternalInput")  # Wv^T [e,d]
    IDN = nc.dram_tensor("ident", [128, 128], F16, kind="ExternalInput")
    INDS = nc.dram_tensor("inds", [128, 4], BF16, kind="ExternalInput")  # l -> l//32 one-hot
    ONES2 = nc.dram_tensor("ones2", [128, 2], BF16, kind="ExternalInput")
    VK = nc.dram_tensor("vk", [D], F32, kind="ExternalInput")        # Wk^T bq
    VQ = nc.dram_tensor("vq", [D], F32, kind="ExternalInput")        # Wq^T bq
    VQ2 = nc.dram_tensor("vq2", [D, 2], F16, kind="ExternalInput")   # [(Wq^T bq)/32, 0]
    BCONST = nc.dram_tensor("bconst", [128], F32, kind="ExternalInput")  # |bq|^2-SHIFT2
    BVB = nc.dram_tensor("bvb", [128, D], BF16, kind="ExternalInput")     # bv bcast
    OUT = nc.dram_tensor("out", [L, D], F16, kind="ExternalOutput")

    from contextlib import ExitStack
    with nc.allow_low_precision("16-bit matmul operands"), \
         tile.TileContext(nc) as tc, ExitStack() as stack:
        sb = stack.enter_context(tc.tile_pool(name="sb", bufs=1))
        e1p = stack.enter_context(tc.tile_pool(name="e1p", bufs=3))
        e1tp = stack.enter_context(tc.tile_pool(name="e1tp", bufs=3))
        e2p = stack.enter_context(tc.tile_pool(name="e2p", bufs=3))
        outp = stack.enter_context(tc.tile_pool(name="outp", bufs=3))
        iv2p = stack.enter_context(tc.tile_pool(name="iv2p", bufs=3))
        # PSUM (8 banks): xT 2 + xp 1 + hg 1 | pass1: s1 2 + m1 1 |
        # pass2: s2 2 + out 3 + rs2 1
        psA = stack.enter_context(tc.tile_pool(name="psA", bufs=3, space="PSUM"))
        psB = stack.enter_context(tc.tile_pool(name="psB", bufs=4, space="PSUM"))
        psC = stack.enter_context(tc.tile_pool(name="psC", bufs=1, space="PSUM"))

        # ---- ACT table warmup ------------------------------------------------
        warm = sb.tile([128, 1], F32)
        nc.vector.memset(warm, 0.0)
        nc.scalar.activation(out=warm, in_=warm,
                             func=mybir.ActivationFunctionType.Exp,
                             bias=warm, scale=1.0)

        # ---- input DMAs (order = DMA_ENGINES order) --------------------------
        ident = sb.tile([128, 128], F16)
        nc.gpsimd.dma_start(out=ident, in_=IDN[:, :])        # SWDGE, off HWDGE
        inds = sb.tile([128, 4], BF16)
        nc.gpsimd.dma_start(out=inds, in_=INDS[:, :])
        x_sb = sb.tile([128, KT, L], F16)
        xr = X.rearrange("(k p) l -> p k l", p=128)
        for ch in range(NCH):
            nc.sync.dma_start(out=x_sb[:, :, bass.ts(ch, 512)],
                              in_=xr[:, :, bass.ts(ch, 512)])
        mks = sb.tile([128, KT, D], F16)
        nc.sync.dma_start(out=mks, in_=MKS.rearrange("(k p) e -> p k e", p=128))
        vk = sb.tile([128, KT], F32)
        nc.gpsimd.dma_start(out=vk, in_=VK.rearrange("(k p) -> p k", p=128))
        vq = sb.tile([128, KT], F32)
        nc.gpsimd.dma_start(out=vq, in_=VQ.rearrange("(k p) -> p k", p=128))
        vq2 = sb.tile([128, KT, 2], F16)
        nc.gpsimd.dma_start(out=vq2, in_=VQ2.rearrange("(k p) t -> p k t", p=128))
        bconst = sb.tile([128, 1], F32)
        nc.gpsimd.dma_start(out=bconst, in_=BCONST.rearrange("(p o) -> p o", o=1))
        ones2 = sb.tile([128, 2], BF16)
        nc.gpsimd.dma_start(out=ones2, in_=ONES2[:, :])
        bvb = sb.tile([128, D], BF16)
        nc.gpsimd.dma_start(out=bvb, in_=BVB[:, :])
        mqs = sb.tile([128, KT, D], F16)
        nc.sync.dma_start(out=mqs, in_=MQS.rearrange("(k p) e -> p k e", p=128))
        wvt = sb.tile([128, KT, D], BF16)
        nc.sync.dma_start(out=wvt, in_=WVT.rearrange("(k p) e -> p k e", p=128))
        sh1 = sb.tile([128, 1], F32)
        nc.vector.memset(sh1, -SHIFT1)

        # ---- x.T tiles (PE transpose) + pooling (tiny PE matmuls) ------------
        xt = sb.tile([128, NLT, D], BF16)
        xpps = psC.tile([128, KT, 128], F32, tag="c")

        xp = sb.tile([128, KT, 128], F16)

        def pool_mm(jp):
            for h in range(2):
                j = 2 * jp + h
                for t in range(KT):
                    nc.tensor.matmul(xpps[:, t, 4 * j:4 * j + 4],
                                     xt[:, j, bass.ts(t, 128)], inds,
                                     start=True, stop=True)
            if jp % 2 == 1:
                ch = (jp - 1) // 2
                nc.scalar.copy(xp[:, :, 16 * ch:16 * ch + 16],
                               xpps[:, :, 16 * ch:16 * ch + 16])

        for jp in range(NLT // 2):
            tp = psB.tile([128, 2, 512], F16, tag="b")
            for h in range(2):
                j = 2 * jp + h
                for k in range(KT):
                    nc.tensor.transpose(tp[:, h, bass.ts(k, 128)],
                                        x_sb[:, k, bass.ts(j, 128)], ident)
            if jp % 2 == 0:
                nc.vector.tensor_copy(xt[:, 2 * jp:2 * jp + 2, :], tp)
            else:
                nc.scalar.copy(xt[:, 2 * jp:2 * jp + 2, :], tp)
            if jp >= 4:
                pool_mm(jp - 4)
        for jq in range(NLT // 2 - 4, NLT // 2):
            pool_mm(jq)
        # ---- H, G, hq (copies on DVE to keep Act free) -----------------------
        h_sb = sb.tile([128, KT, 128], F16)
        hps = psA.tile([128, KT, 128], F32, tag="a")
        for et in range(KT):
            for ck in range(KT):
                nc.tensor.matmul(hps[:, et, :],
                                 mks[:, ck, bass.ts(et, 128)], xp[:, ck, :],
                                 start=(ck == 0), stop=(ck == KT - 1))
        for et in range(KT):
            nc.scalar.activation(out=h_sb[:, et, :], in_=hps[:, et, :],
                                 func=mybir.ActivationFunctionType.Identity,
                                 bias=vk[:, et:et + 1], scale=1.0)
        hqps = psA.tile([128, 2], F32, tag="a")
        for ck in range(KT):
            nc.tensor.matmul(hqps, xp[:, ck, :], vq2[:, ck, :],
                             start=(ck == 0), stop=(ck == KT - 1))
        hq = sb.tile([128, 1], F32)
        nc.scalar.activation(out=hq, in_=hqps[:, 0:1],
                             func=mybir.ActivationFunctionType.Identity,
                             bias=bconst, scale=1.0)
        g_sb = sb.tile([128, KT, 128], F16)
        gps = psA.tile([128, KT, 128], F32, tag="a")
        for et in range(KT):
            for ck in range(KT):
                nc.tensor.matmul(gps[:, et, :],
                                 mqs[:, ck, bass.ts(et, 128)], xp[:, ck, :],
                                 start=(ck == 0), stop=(ck == KT - 1))
        for et in range(KT):
            nc.scalar.activation(out=g_sb[:, et, :], in_=gps[:, et, :],
                                 func=mybir.ActivationFunctionType.Identity,
                                 bias=vq[:, et:et + 1], scale=1.0)

        # ---- pass 1: S1 -> E1 -> (xbar) E1T -> M1, software-pipelined --------
        # M1 for chunk a is issued after S1 for chunk a+2, so the PE never
        # stalls on the exp + xbar-transpose round trip.
        rs1 = sb.tile([128, NCH], F32)
        m1tps = []
        for ek in range(KT):
            m1b = psB.tile([128, 128], F32, tag="b")
            m1tps.append(m1b)
        e1ts = []

        def s1_stage(a):
            s1 = psA.tile([128, 512], F32, tag="a")
            for k in range(KT):
                nc.tensor.matmul(s1, h_sb[:, k, :], x_sb[:, k, bass.ts(a, 512)],
                                 start=(k == 0), stop=(k == KT - 1))
            e1 = e1p.tile([128, 512], BF16, tag="e1")
            nc.scalar.activation(out=e1, in_=s1,
                                 func=mybir.ActivationFunctionType.Exp,
                                 bias=sh1, scale=1.0,
                                 accum_out=rs1[:, a:a + 1])
            e1t = e1tp.tile([128, 4, 128], BF16, tag="e1t")
            nc.sync.dma_start_transpose(e1t, e1)
            e1ts.append(e1t)

        def m1_stage(a):
            e1t = e1ts[a]
            for u in range(4):
                j = 4 * a + u
                for ek in range(KT):
                    nc.tensor.matmul(m1tps[ek],
                                     xt[:, j, bass.ts(ek, 128)], e1t[:, u, :],
                                     start=(j == 0), stop=(j == NLT - 1))

        s1_stage(0)
        s1_stage(1)
        for a in range(NCH):
            if a + 2 < NCH:
                s1_stage(a + 2)
            m1_stage(a)

        # ---- pass 2: S2 -> E2 -> out = (E2^T cbv) * inv2 ---------------------
        # ident (fp16) reused as transpose stationary; e2 slices are the
        # stationary lhs of the output matmuls (contraction over p needs no
        # transpose), ones2 column rides along for colsum2.
        or_ = OUT.rearrange("(c j p) d -> c p j d", j=4, p=128)
        e2s = []

        def s2_stage(a):
            s2 = psA.tile([128, 512], F32, tag="a")
            for k in range(KT):
                nc.tensor.matmul(s2, g_sb[:, k, :], x_sb[:, k, bass.ts(a, 512)],
                                 start=(k == 0), stop=(k == KT - 1))
            e2 = e2p.tile([128, 512], BF16, tag="e2")
            nc.scalar.activation(out=e2, in_=s2,
                                 func=mybir.ActivationFunctionType.Exp,
                                 bias=hq, scale=1.0)
            e2s.append(e2)

        inv2s = []

        def rs2_stage(a):
            e2 = e2s[a]
            rsps = psC.tile([128, 4, 2], F32, tag="c")
            for u in range(4):
                nc.tensor.matmul(rsps[:, u, :], e2[:, bass.ts(u, 128)], ones2,
                                 start=True, stop=True)
            inv2 = iv2p.tile([128, 4], F32, tag="iv2")
            nc.vector.reciprocal(inv2, rsps[:, :, 0])
            inv2s.append(inv2)

        def out_stage(a):
            e2 = e2s[a]
            inv2 = inv2s[a]
            o_sb = outp.tile([128, 4, D], F16, tag="o")
            for u in range(4):
                ops = psB.tile([128, D], F32, tag="b")
                nc.tensor.matmul(ops, e2[:, bass.ts(u, 128)], cbv,
                                 start=True, stop=True)
                on_act = (u == 0) or (u == 2 and a % 4 != 1)
                if on_act:
                    nc.scalar.activation(
                        out=o_sb[:, u, :], in_=ops,
                        func=mybir.ActivationFunctionType.Identity,
                        bias=0.0, scale=inv2[:, u:u + 1])
                else:
                    nc.vector.tensor_scalar_mul(o_sb[:, u, :], ops,
                                                inv2[:, u:u + 1])
            nc.sync.dma_start(out=or_[a], in_=o_sb)

        s2_stage(0)
        s2_stage(1)
        rs2_stage(0)
        rsum1 = sb.tile([128, 1], F32)
        nc.vector.reduce_sum(out=rsum1, in_=rs1, axis=mybir.AxisListType.X)
        inv1 = sb.tile([128, 1], F32)
        nc.vector.reciprocal(inv1, rsum1)

        # ---- c = (M1 @ Wv^T)/rowsum1 + bv ------------------------------------
        m1t = sb.tile([128, KT, 128], BF16)
        for ek in range(KT):
            if ek % 2 == 0:
                nc.scalar.copy(m1t[:, ek, :], m1tps[ek])
            else:
                nc.vector.tensor_copy(m1t[:, ek, :], m1tps[ek])
        cps = psA.tile([128, D], F32, tag="a")
        for i in range(KT):
            nc.tensor.matmul(cps, m1t[:, i, :], wvt[:, i, :],
                             start=(i == 0), stop=(i == KT - 1))
        cbv = sb.tile([128, D], BF16)
        nc.vector.scalar_tensor_tensor(out=cbv, in0=cps, scalar=inv1, in1=bvb,
                                       op0=mybir.AluOpType.mult,
                                       op1=mybir.AluOpType.add)

        for a in range(NCH):
            if a + 2 < NCH:
                s2_stage(a + 2)
            if a + 1 < NCH:
                rs2_stage(a + 1)
            out_stage(a)

    nc.compile()
    return nc


def _host_inputs(x, Wq, bq, Wk, bk, Wv, bv):
    del bk  # stage-1 softmax is invariant to the k-projection bias
    Wq = np.asarray(Wq, dtype=np.float32)
    Wk = np.asarray(Wk, dtype=np.float32)
    Wv = np.asarray(Wv, dtype=np.float32)
    bq = np.asarray(bq, dtype=np.float32)
    bv = np.asarray(bv, dtype=np.float32)
    bf16 = ml_dtypes.bfloat16
    inds = np.zeros((128, 4), dtype=np.float32)
    inds[np.arange(128), np.arange(128) // SEG] = 1.0
    ones2 = np.zeros((128, 2), dtype=np.float32)
    ones2[:, 0] = 1.0
    common = {
        "mks": ((Wq.T @ Wk) / SEG).astype(np.float16),
        "mqs": ((Wq.T @ Wq) / SEG).astype(np.float16),
        "wvt": np.ascontiguousarray(Wv.T).astype(bf16),
        "ident": np.eye(128, dtype=np.float16),
        "inds": inds.astype(bf16),
        "ones2": ones2.astype(bf16),
        "vk": (Wk.T @ bq).astype(np.float32),
        "vq": (Wq.T @ bq).astype(np.float32),
        "vq2": np.stack([(Wq.T @ bq) / SEG, np.zeros(D, np.float32)],
                        axis=1).astype(np.float16),
        "bconst": np.full(128, float(bq @ bq) - SHIFT2, dtype=np.float32),
        "bvb": np.tile(bv[None, :], (128, 1)).astype(bf16),
    }
    maps = []
    for b in range(B):
        m = dict(common)
        m["x"] = np.ascontiguousarray(x[b]).astype(np.float16)
        maps.append(m)
    return maps


def kernel(x, Wq, bq, Wk, bk, Wv, bv):
    x = np.asarray(x, dtype=np.float32)
    if "nc" not in _CACHE:
        _CACHE["nc"] = build()
    nc = _CACHE["nc"]
    in_maps = _host_inputs(x, Wq, bq, Wk, bk, Wv, bv)
    res = run_bass_kernel_spmd(nc, in_maps, core_ids=list(range(B)))
    out = np.empty((B, D, L), dtype=np.float32)
    for b in range(B):
        out[b] = np.asarray(res.results[b]["out"]).astype(np.float32).T
    return out
